# revision 1
# baseline (speedup 1.0000x reference)
"""EquivariantEvolution kernel for 8 Trainium2 NeuronCores (Bass/Tile).

Math (per sample, reference):
    alpha = Linear2(silu(Linear1(z)))                     # [NG]
    A     = sum_g alpha_g G_g                             # [D, D]
    z_t   = (I + A + A^2/2 + A^3/6 + A^4/24) z            # order-4 Taylor
    h1    = W1 z_t + b1
    out   = W2 (sigmoid(|h1| + eps) * h1) + b2

Device strategy (pure data-parallel over batch, feature-major layout):
  * Host pre-transposes z to [D, B/8] per core; all weight reshuffling is
    done on host so the device only runs matmuls / elementwise ops.
  * Horner:  v <- z + (1/k) A v.  A v is computed as one K=128 contraction:
      y[(g,i), b] = alpha_g[b] * v[i, b]   (DVE tensor_tensor, alpha
      replicated across the 32 i-partitions by construction)
      (A v)[j, b] = sum_{(g,i)} G[g,j,i] y[(g,i),b]   (two K=128 matmuls)
    The matmul lhsT is tiled 4x along M so the output lands pre-replicated
    [(r,j), b] for the next step's elementwise multiply.
  * sigmoid(x) = 0.5 tanh(x/2) + 0.5 keeps everything in the silu ACT
    table set; the lone sqrt is batched across all tiles (2 table switches
    per kernel instead of 2 per tile).
  * k=1 Horner step is fused with the MLP first layer (W1 folded into the
    Taylor weights); the gate multiply is commuted into a doubled final
    matmul: W2(gate*h1) = 0.5 W2 (t*h1) + 0.5 W2 h1.
"""

import os
import sys

import numpy as np

for _p in ("/opt/trn_rl_repo", "/root/.axon_site/_ro/trn_rl_repo"):
    if os.path.isdir(_p) and _p not in sys.path:
        sys.path.insert(0, _p)

import concourse.bass as bass
import concourse.mybir as mybir
import concourse.tile as tile
from concourse.bass_utils import run_bass_kernel_spmd

B, D, H, NG = 65536, 32, 128, 8
NCORES = 8
BC = B // NCORES          # samples per core
BT = 512                  # samples (free-dim columns) per tile
EPS = 1e-6
F32 = mybir.dt.float32
F32R = mybir.dt.float32r
BF16 = mybir.dt.bfloat16
AF = mybir.ActivationFunctionType

# Taylor weights run as bf16 (full-rate PE, overlappable LDWEIGHTS);
# z-path / extractor / MLP stay float32r for precision.
_BF16_PARAMS = ("LT_t4", "LT_b4", "LT_t3", "LT_b3")


def _param_dt(name):
    if name.startswith("B"):
        return F32
    if name in _BF16_PARAMS:
        return BF16
    return F32R


def _r(ap):
    """View an fp32 AP as float32r for single-pass full-rate PE matmuls."""
    return ap.bitcast(F32R)

# weight/bias DRAM parameters: name -> shape
_PARAM_SHAPES = {
    "LT_h": [D, H],          # W_se1^T
    "LT_At": [H, H],         # W_se2[0:4] replicated over i
    "LT_Ab": [H, H],         # W_se2[4:8] replicated over i
    "Bse1": [H, 1],
    "Bse2t": [H, 1],
    "Bse2b": [H, 1],
    "LT_t4": [H, H], "LT_b4": [H, H],
    "LT_t3": [H, H], "LT_b3": [H, H],
    "LT_t2": [H, H], "LT_b2k": [H, H],
    "LT_t1": [H, H], "LT_b1k": [H, H],   # k=1 step fused with W1
    "LT_z": [D, H],          # identity replicated 4x along M
    "LT_W1z": [D, H],        # W1^T
    "B1": [H, 1],
    "LT_W2": [H, D],         # 0.5 * W2^T
    "B2": [D, 1],
}


def _split_multi_waits(nc, max_waits=1):
    """This toolchain's walrus rejects >1 sync-wait on an instruction
    ("Too many sync wait commands"); hoist extra waits onto preceding
    same-engine NOPs (in-order engines make this semantics-preserving)."""
    n_new = 0
    for f in nc.m.functions:
        for bb in f.blocks:
            out = []
            for ins in bb.instructions:
                si = getattr(ins, "sync_info", None)
                if si is not None and si.on_wait and len(si.on_wait) > max_waits:
                    waits = list(si.on_wait)
                    chunks = [waits[i:i + max_waits] for i in range(0, len(waits), max_waits)]
                    for ci, ch in enumerate(chunks[:-1]):
                        nop = mybir.InstNoOp(
                            name=f"{ins.name}-wsplit{ci}",
                            engine=ins.engine,
                            sync_info=mybir.SyncInfo(on_wait=ch, on_update=[]),
                            bass_nofuse=True,
                        )
                        out.append(nop)
                        n_new += 1
                    ins.sync_info = mybir.SyncInfo(on_wait=chunks[-1], on_update=si.on_update)
                out.append(ins)
            bb.instructions[:] = out
    return n_new


def _build_program(bc: int, sim_safe: bool = False, split_waits: bool = True):
    """Trace the per-core Bass program for bc samples.

    sim_safe decomposes Silu into Sigmoid*x (CoreSim has no Silu handler);
    the hardware path uses the native Silu LUT.
    """
    nt = bc // BT
    nc = bass.Bass()

    zT = nc.declare_dram_parameter("zT", [D, bc], F32R, isOutput=False)
    params = {
        name: nc.declare_dram_parameter(name, shape, _param_dt(name), isOutput=False)
        for name, shape in _PARAM_SHAPES.items()
    }
    eall = nc.declare_dram_parameter("E_all", [nt, nt * H], F32R, isOutput=False)
    onsq = nc.declare_dram_parameter("ONES_nsq", [H, nt * nt], F32R, isOutput=False)
    outT = nc.declare_dram_parameter("outT", [D, bc], F32, isOutput=True)

    with tile.TileContext(nc) as tc:
        with (
            tc.tile_pool(name="consts", bufs=1) as consts,
            tc.tile_pool(name="zv4", bufs=4) as zv4_pool,
            tc.tile_pool(name="hs", bufs=3) as hs_pool,
            tc.tile_pool(name="acat", bufs=3) as acat_pool,
            tc.tile_pool(name="ycat", bufs=3) as ycat_pool,
            tc.tile_pool(name="sq", bufs=3) as sq_pool,
            tc.tile_pool(name="h1s", bufs=nt) as h1s_pool,
            tc.tile_pool(name="gate", bufs=1) as gate_pool,
            tc.tile_pool(name="a1g", bufs=2) as a1g_pool,
            tc.tile_pool(name="outs", bufs=3) as outs_pool,
            tc.tile_pool(name="ps", bufs=2, space=bass.MemorySpace.PSUM) as ps_pool,
            tc.tile_pool(name="warm", bufs=1, space=bass.MemorySpace.PSUM) as warm_pool,
            tc.tile_pool(name="pv", bufs=3, space=bass.MemorySpace.PSUM) as pv_pool,
            tc.tile_pool(name="psn", bufs=1, space=bass.MemorySpace.PSUM) as psn_pool,
            tc.tile_pool(name="pso", bufs=1, space=bass.MemorySpace.PSUM) as pso_pool,
        ):
            # ---- load constants into SBUF ----
            ct = {}
            for name, shape in _PARAM_SHAPES.items():
                t = consts.tile(shape, _param_dt(name), name=f"c_{name}")
                nc.sync.dma_start(t[:], params[name][:])
                ct[name] = t
            e_t = consts.tile([nt, nt * H], F32R, name="c_E")
            nc.sync.dma_start(e_t[:], eall[:])
            onsq_t = consts.tile([H, nt * nt], F32R, name="c_onsq")
            nc.sync.dma_start(onsq_t[:], onsq[:])
            zero_b = consts.tile([nt, 1], F32, name="zero_b")
            nc.vector.memset(zero_b[:], 0.0)
            tanh_b = consts.tile([nt, 1], F32, name="tanh_b")
            nc.vector.memset(tanh_b[:], 0.5 * EPS)

            taylor = [
                (ct["LT_t4"], ct["LT_b4"], BF16),
                (ct["LT_t3"], ct["LT_b3"], BF16),
                (ct["LT_t2"], ct["LT_b2k"], F32R),
            ]

            h1s_tiles = []
            nsq_ps = psn_pool.tile([nt, BT], F32, name="nsq_ps", tag="nsq")

            # ---- HAM warm-up: ~17us of dense matmuls pushes the PE clock
            # gate to K=8/8 (2.4 GHz); the main stream's gaps are short
            # enough (<3.4us) to keep it there ----
            wscr = consts.tile([H, BT], BF16, name="wscr")
            nc.vector.memset(wscr[:], 0.0)
            wps = warm_pool.tile([H, BT], F32, name="wps", tag="warm")

            def warm(n, cols=BT):
                for _ in range(n):
                    nc.tensor.matmul(wps[:, 0:cols], ct["LT_t4"][:], wscr[:, 0:cols],
                                     start=True, stop=True)

            warm(64)

            # ================= phase A =================
            # tiles are emitted in pairs, stage-interleaved, so the PE always
            # has the partner tile's matmuls to run while DVE does the y-muls
            for tp in range(0, nt, 2):
                if tp in (6, 12):
                    warm(28)   # re-fire K=8/8 after the ~65us warm budget
                pair = [t for t in (tp, tp + 1) if t < nt]
                zv4s, hss, acats, ycats = {}, {}, {}, {}

                for t in pair:
                    zv4 = zv4_pool.tile([H, BT], F32R, name="zv4")
                    for r in range(4):
                        nc.gpsimd.dma_start(zv4[32 * r:32 * (r + 1), :], zT[:, bass.ts(t, BT)])
                    zv4s[t] = zv4

                for t in pair:
                    hp = ps_pool.tile([H, BT], F32, name="hp", tag="ps")
                    nc.tensor.matmul(hp[:], ct["LT_h"][:], zv4s[t][0:D, :], start=True, stop=True)
                    hs = hs_pool.tile([H, BT], F32R, name="hs")
                    if sim_safe:
                        sg = hs_pool.tile([H, BT], F32, name="sg")
                        nc.scalar.activation(sg[:], hp[:], AF.Sigmoid, bias=ct["Bse1"][:])
                        hx = hs_pool.tile([H, BT], F32, name="hx")
                        nc.scalar.activation(hx[:], hp[:], AF.Identity, bias=ct["Bse1"][:])
                        nc.vector.tensor_tensor(hs[:], sg[:], hx[:], mybir.AluOpType.mult)
                    else:
                        nc.scalar.activation(hs[:], hp[:], AF.Silu, bias=ct["Bse1"][:])
                    hss[t] = hs

                for t in pair:
                    apt = ps_pool.tile([H, BT], F32, name="apt", tag="ps")
                    nc.tensor.matmul(apt[:], ct["LT_At"][:], hss[t][:], start=True, stop=True)
                    apb = ps_pool.tile([H, BT], F32, name="apb", tag="ps")
                    nc.tensor.matmul(apb[:], ct["LT_Ab"][:], hss[t][:], start=True, stop=True)
                    acat = acat_pool.tile([H, 2, BT], F32, name="acat")
                    nc.scalar.activation(acat[:, 0, :], apt[:], AF.Identity, bias=ct["Bse2t"][:])
                    nc.scalar.activation(acat[:, 1, :], apb[:], AF.Identity, bias=ct["Bse2b"][:])
                    acats[t] = acat

                warm(3, 256)
                for t in pair:
                    ycat = ycat_pool.tile([H, 2, BT], BF16, name="ycat0")
                    nc.vector.tensor_tensor(
                        ycat[:], acats[t][:],
                        zv4s[t][:, None, :].broadcast_to([H, 2, BT]),
                        mybir.AluOpType.mult,
                    )
                    ycats[t] = ycat

                for step_i, (lt_top, lt_bot, ydt) in enumerate(taylor):
                    pvs = {}
                    for t in pair:
                        pv = pv_pool.tile([H, BT], F32, name="pv", tag="pv")
                        nc.tensor.matmul(pv[:], lt_top[:], ycats[t][:, 0, :], start=True, stop=False)
                        nc.tensor.matmul(pv[:], lt_bot[:], ycats[t][:, 1, :], start=False, stop=False)
                        nc.tensor.matmul(pv[:], ct["LT_z"][:], zv4s[t][0:D, :], start=False, stop=True)
                        pvs[t] = pv
                    warm(3, 256)
                    nxt = taylor[step_i + 1][2] if step_i + 1 < len(taylor) else F32R
                    for t in pair:
                        ycat = ycat_pool.tile([H, 2, BT], nxt, name="ycat")
                        nc.vector.tensor_tensor(
                            ycat[:], acats[t][:],
                            pvs[t][:, None, :].broadcast_to([H, 2, BT]),
                            mybir.AluOpType.mult,
                        )
                        ycats[t] = ycat

                warm(2, 256)
                h1ps = {}
                for t in pair:
                    h1p = ps_pool.tile([H, BT], F32, name="h1p", tag="ps")
                    nc.tensor.matmul(h1p[:], ct["LT_t1"][:], ycats[t][:, 0, :], start=True, stop=False)
                    nc.tensor.matmul(h1p[:], ct["LT_b1k"][:], ycats[t][:, 1, :], start=False, stop=False)
                    nc.tensor.matmul(h1p[:], ct["LT_W1z"][:], zv4s[t][0:D, :], start=False, stop=True)
                    h1ps[t] = h1p

                warm(2, 256)
                for t in pair:
                    h1s = h1s_pool.tile([H, BT], F32R, name="h1s")
                    nc.scalar.activation(h1s[:], h1ps[t][:], AF.Identity, bias=ct["B1"][:])
                    sq = sq_pool.tile([H, BT], F32R, name="sq")
                    nc.scalar.activation(sq[:], h1ps[t][:], AF.Square, bias=ct["B1"][:])
                    h1s_tiles.append(h1s)
                    nc.tensor.matmul(
                        nsq_ps[:], onsq_t[:, bass.ts(t, nt)], sq[:],
                        start=(t == 0), stop=(t == nt - 1), skip_group_check=True,
                    )

            # ============== gate (batched sqrt + tanh) ==============
            warm(14)
            rt_all = gate_pool.tile([nt, BT], F32, name="rt_all")
            nc.scalar.activation(rt_all[:], nsq_ps[:], AF.Sqrt, bias=zero_b[:])
            t_all = gate_pool.tile([nt, BT], F32R, name="t_all")
            # sigmoid(norm + eps) = 0.5 tanh(0.5 norm + eps/2) + 0.5
            nc.scalar.activation(t_all[:], rt_all[:], AF.Tanh, bias=tanh_b[:], scale=0.5)

            # ================= phase B =================
            for t in range(nt):
                sl = bass.ts(t, BT)
                trp = pso_pool.tile([H, BT], F32, name="trp", tag="pso")
                nc.tensor.matmul(
                    trp[:], e_t[:, bass.ts(t, H)], t_all[:], start=True, stop=True
                )
                a1g = a1g_pool.tile([H, BT], F32R, name="a1g")
                nc.vector.tensor_tensor(
                    a1g[:], h1s_tiles[t][:], trp[:], mybir.AluOpType.mult
                )
                outp = pso_pool.tile([D, BT], F32, name="outp", tag="pso")
                nc.tensor.matmul(outp[:], ct["LT_W2"][:], a1g[:], start=True, stop=False)
                nc.tensor.matmul(outp[:], ct["LT_W2"][:], h1s_tiles[t][:], start=False, stop=True)
                warm(2, 256)
                outs = outs_pool.tile([D, BT], F32, name="outs")
                nc.scalar.activation(outs[:], outp[:], AF.Identity, bias=ct["B2"][:])
                nc.sync.dma_start(outT[:, sl], outs[:])

    if split_waits:
        _split_multi_waits(nc)
    return nc


def _host_params(G, W_se1, b_se1, W_se2, b_se2, W1, b1, W2, b2, nt):
    f = np.float32
    G = np.asarray(G, f)
    Gflat = np.transpose(G, (0, 2, 1)).reshape(NG * D, D)  # [(g,i), j] = G[g,j,i]
    W1G = Gflat @ np.asarray(W1, f).T                      # [(g,i), m]
    p = {
        "LT_h": np.asarray(W_se1, f).T,
        "LT_At": np.repeat(np.asarray(W_se2, f).T[:, 0:4], 32, axis=1),
        "LT_Ab": np.repeat(np.asarray(W_se2, f).T[:, 4:8], 32, axis=1),
        "Bse1": np.asarray(b_se1, f).reshape(H, 1),
        "Bse2t": np.repeat(np.asarray(b_se2, f)[0:4], 32).reshape(H, 1),
        "Bse2b": np.repeat(np.asarray(b_se2, f)[4:8], 32).reshape(H, 1),
        "LT_z": np.tile(np.eye(D, dtype=f), (1, 4)),
        "LT_W1z": np.asarray(W1, f).T,
        "B1": np.asarray(b1, f).reshape(H, 1),
        "LT_W2": 0.5 * np.asarray(W2, f).T,
        "B2": np.asarray(b2, f).reshape(D, 1),
        "LT_t1": np.ascontiguousarray(W1G[:H]),
        "LT_b1k": np.ascontiguousarray(W1G[H:]),
    }
    for k, tname, bname in ((4, "LT_t4", "LT_b4"), (3, "LT_t3", "LT_b3"), (2, "LT_t2", "LT_b2k")):
        scaled = np.tile(Gflat * f(1.0 / k), (1, 4))
        p[tname] = np.ascontiguousarray(scaled[:H])
        p[bname] = np.ascontiguousarray(scaled[H:])
    p["E_all"] = np.ascontiguousarray(np.repeat(np.eye(nt, dtype=f), H, axis=1))
    import ml_dtypes
    for name in _BF16_PARAMS:
        p[name] = p[name].astype(ml_dtypes.bfloat16)
    p["ONES_nsq"] = np.ascontiguousarray(np.tile(np.eye(nt, dtype=f).reshape(1, nt * nt), (H, 1)))
    return p


def _run(z, G, W_se1, b_se1, W_se2, b_se2, W1, b1, W2, b2, trace=False, **trace_kw):
    z = np.asarray(z, np.float32)
    nt = BC // BT
    params = _host_params(G, W_se1, b_se1, W_se2, b_se2, W1, b1, W2, b2, nt)

    # shard: per-core feature-major slices
    zT = np.ascontiguousarray(z.reshape(NCORES, BC, D).transpose(0, 2, 1))

    nc = _build_program(BC)
    in_maps = [{"zT": zT[c], **params} for c in range(NCORES)]
    res = run_bass_kernel_spmd(nc, in_maps, list(range(NCORES)), trace=trace, **trace_kw)

    outT = np.stack([res.results[c]["outT"] for c in range(NCORES)])
    out = outT.transpose(0, 2, 1).reshape(B, D)
    return np.ascontiguousarray(out.astype(np.float32)), res


def kernel(z, G, W_se1, b_se1, W_se2, b_se2, W1, b1, W2, b2):
    out, _ = _run(z, G, W_se1, b_se1, W_se2, b_se2, W1, b1, W2, b2, trace=False)
    return out


if __name__ == "__main__":
    rng = np.random.default_rng(0)
    inputs = {
        "z": rng.standard_normal((B, D), dtype=np.float32),
        "G": (rng.standard_normal((NG, D, D)) * 0.1).astype(np.float32),
        "W_se1": (rng.standard_normal((H, D)) / np.sqrt(D)).astype(np.float32),
        "b_se1": np.zeros(H, np.float32),
        "W_se2": (rng.standard_normal((NG, H)) / np.sqrt(H)).astype(np.float32),
        "b_se2": np.zeros(NG, np.float32),
        "W1": (rng.standard_normal((H, D)) * 0.01).astype(np.float32),
        "b1": np.zeros(H, np.float32),
        "W2": (rng.standard_normal((D, H)) * 0.01).astype(np.float32),
        "b2": np.zeros(D, np.float32),
    }
    out = kernel(**inputs)
    print("kernel output", out.shape, out.dtype, float(np.abs(out).max()))



# revision 10
# speedup vs baseline: 1.3253x; 1.3253x over previous
"""EquivariantEvolution kernel for 8 Trainium2 NeuronCores (Bass/Tile).

Math (per sample):
    alpha = W_se2 silu(W_se1 z + b_se1) + b_se2            # [NG=8]
    A     = sum_g alpha_g G_g                              # [32, 32]
    z_t   = (I + A + A^2/2 + A^3/6 + A^4/24) z             # order-4 Taylor
    h1    = W1 z_t + b1                                    # [128]
    out   = sigmoid(|h1| + eps) * (W2 h1) + b2             # gate commuted past W2

Device strategy (pure batch data-parallel, feature-major [feat, samples]):
  * Horner: v <- z + (1/k) A v.  A v as y[(g,i),b] = alpha_g[b] v_i[b]
    (elementwise outer product) contracted by two K=128 matmuls whose
    lhsT is pre-replicated 4x along M so outputs land ready for the next
    elementwise step.  The +z fold is a K=32 matmul row-packed 2x via
    tile_position so pairs run concurrently in the PE array.
  * Everything the PE touches is bf16 (FWL weight loads, half DMA);
    all accumulation stays f32 in PSUM.
  * The norm-squared reduction for all 16 tiles accumulates into ONE
    PSUM bank (4 col-groups x 4 rows), so the sqrt/tanh run once per
    kernel (2 ACT table switches total).
  * Phase B: gate = 0.5*tanh+0.5 is broadcast to 4 tiles at once by a
    single 0/0.5 matmul; W2 h1 is col-packed 4x (M=32 each); one DVE
    multiply + one 4-strip DMA store finishes 4 tiles.
  * No HAM warm-up spam: a short burst of zero matmuls during the
    initial parameter DMAs brings the clock to K=8/8; after that the
    real matmul stream is dense enough to keep it there.
"""

import os
import sys

import numpy as np

for _p in ("/opt/trn_rl_repo", "/root/.axon_site/_ro/trn_rl_repo"):
    if os.path.isdir(_p) and _p not in sys.path:
        sys.path.insert(0, _p)

import concourse.bass as bass
import concourse.mybir as mybir
import concourse.tile as tile
from concourse.bass_utils import run_bass_kernel_spmd

B, D, H, NG = 65536, 32, 128, 8
NCORES = 8
BC = B // NCORES          # samples per core
BT = 512                  # samples per tile (PSUM bank width in f32)
GS = 4                    # tiles per group
EPS = 1e-6
F32 = mybir.dt.float32
F32R = mybir.dt.float32r
BF16 = mybir.dt.bfloat16
AF = mybir.ActivationFunctionType

# whether the z-path outer product (all-SBUF operands) runs on gpsimd
YCAT0_ON_GPSIMD = True


def _split_multi_waits(nc, max_waits=1):
    """This toolchain's walrus rejects >1 sync-wait on an instruction
    ("Too many sync wait commands"); hoist extra waits onto preceding
    same-engine NOPs (in-order engines make this semantics-preserving)."""
    n_new = 0
    for f in nc.m.functions:
        for bb in f.blocks:
            out = []
            for ins in bb.instructions:
                si = getattr(ins, "sync_info", None)
                if si is not None and si.on_wait and len(si.on_wait) > max_waits:
                    waits = list(si.on_wait)
                    chunks = [waits[i:i + max_waits] for i in range(0, len(waits), max_waits)]
                    for ci, ch in enumerate(chunks[:-1]):
                        nop = mybir.InstNoOp(
                            name=f"{ins.name}-wsplit{ci}",
                            engine=ins.engine,
                            sync_info=mybir.SyncInfo(on_wait=ch, on_update=[]),
                            bass_nofuse=True,
                        )
                        out.append(nop)
                        n_new += 1
                    ins.sync_info = mybir.SyncInfo(on_wait=chunks[-1], on_update=si.on_update)
                out.append(ins)
            bb.instructions[:] = out
    return n_new


# DRAM parameters: name -> (shape, dtype).  All matmul operands bf16.
_PARAM_SHAPES = {
    "LT_h4": ([H, H], BF16),      # W_se1^T tiled 4x along partitions
    "LT_At": ([H, H], BF16),      # W_se2[0:4] replicated 32x over M
    "LT_Ab": ([H, H], BF16),      # W_se2[4:8]
    "Bse1": ([H, 1], F32),
    "Bse2t": ([H, 1], F32),
    "Bse2b": ([H, 1], F32),
    "LT_t4": ([H, H], BF16), "LT_b4": ([H, H], BF16),
    "LT_t3": ([H, H], BF16), "LT_b3": ([H, H], BF16),
    "LT_t2": ([H, H], BF16), "LT_b2": ([H, H], BF16),
    "LT_t1": ([H, H], BF16), "LT_b1": ([H, H], BF16),   # W1-folded k=1 step
    "LT_z4": ([H, H], BF16),      # I32 tiled (4,4): row-packable +z fold
    "LT_w1z4": ([H, H], BF16),    # W1^T tiled 4x along partitions
    "B1": ([H, 1], F32),
    "onsq4": ([H, H], BF16),      # norm-sq row-select weights (variant r at cols 32r)
    "E4": ([H, 4 * H], BF16),     # 0.5-scaled gate broadcast, one [H,H] block per group
    "LT_w2c": ([H, H], BF16),     # W2^T tiled 4x along M (col-packable)
    "B2r": ([H, 1], F32),         # b2 tiled 4x along partitions
}


def _build_program(bc: int, zero_bias: bool, sim_safe: bool = False, split_waits: bool = True):
    nt = bc // BT
    ngrp = nt // GS
    nc = bass.Bass()

    zT = nc.declare_dram_parameter("zT", [D, bc], BF16, isOutput=False)
    params = {
        name: nc.declare_dram_parameter(name, shape, dt, isOutput=False)
        for name, (shape, dt) in _PARAM_SHAPES.items()
    }
    outT = nc.declare_dram_parameter("outT", [D, bc], F32, isOutput=True)

    with tile.TileContext(nc) as tc:
        with (
            tc.tile_pool(name="consts", bufs=1) as consts,
            tc.tile_pool(name="zv4", bufs=2) as zv4_pool,
            tc.tile_pool(name="hs", bufs=3) as hs_pool,
            tc.tile_pool(name="acat", bufs=8) as acat_pool,
            tc.tile_pool(name="ycat", bufs=8) as ycat_pool,
            tc.tile_pool(name="sq", bufs=5) as sq_pool,
            tc.tile_pool(name="h1s", bufs=nt) as h1s_pool,
            tc.tile_pool(name="gate", bufs=4) as gate_pool,
            tc.tile_pool(name="og", bufs=3) as og_pool,
            # PSUM: pv(4) + ap(2) + nsq(1) + pso(1) = 8 banks
            tc.tile_pool(name="pv", bufs=4, space=bass.MemorySpace.PSUM) as pv_pool,
            tc.tile_pool(name="ap", bufs=1, space=bass.MemorySpace.PSUM) as ap_pool,
            tc.tile_pool(name="psn", bufs=1, space=bass.MemorySpace.PSUM) as psn_pool,
            tc.tile_pool(name="pso", bufs=1, space=bass.MemorySpace.PSUM) as pso_pool,
        ):
            # ---- constants into SBUF ----
            ct = {}
            for name, (shape, dt) in _PARAM_SHAPES.items():
                t = consts.tile(shape, dt, name=f"c_{name}")
                nc.sync.dma_start(t[:], params[name][:])
                ct[name] = t
            half_b = consts.tile([H, 1], F32, name="half_b")
            nc.vector.memset(half_b[:], 0.5)
            zero_b = consts.tile([H, 1], F32, name="zero_b")
            nc.vector.memset(zero_b[:], 0.0)
            tanh_b = consts.tile([H, 1], F32, name="tanh_b")
            nc.vector.memset(tanh_b[:], 0.5 * EPS)

            # ---- HAM ramp: zero-matmuls run while parameter DMAs land ----
            wscr = consts.tile([H, BT], BF16, name="wscr")
            nc.vector.memset(wscr[:], 0.0)
            wps = pso_pool.tile([H, BT], F32, name="wps", tag="pso")
            for _ in range(12):
                nc.tensor.matmul(wps[:], wscr[:, 0:H], wscr[:], start=True, stop=True)

            taylor = [
                (ct["LT_t4"], ct["LT_b4"]),
                (ct["LT_t3"], ct["LT_b3"]),
                (ct["LT_t2"], ct["LT_b2"]),
            ]

            nsq_ps = psn_pool.tile([H, BT], F32, name="nsq_ps", tag="nsq")
            h1s_tiles = []

            # ================= phase A =================
            for g in range(ngrp):
                zv4 = zv4_pool.tile([H, GS, BT], BF16, name="zv4")
                for s in range(4):
                    nc.gpsimd.dma_start(
                        zv4[32 * s:32 * (s + 1), :, :],
                        zT[:, bass.ts(g, GS * BT)],
                    )

                # ---- strength extractor for the 4 tiles ----
                acats, ycats = {}, {}
                for j in range(GS):
                    hp = pv_pool.tile([H, BT], F32, name="hp", tag="pv")
                    nc.tensor.matmul(hp[:], ct["LT_h4"][0:D, :], zv4[0:D, j, :],
                                     start=True, stop=True)
                    hs = hs_pool.tile([H, BT], BF16, name="hs")
                    if sim_safe:
                        sg = hs_pool.tile([H, BT], F32, name="sg")
                        nc.scalar.activation(sg[:], hp[:], AF.Sigmoid, bias=ct["Bse1"][:])
                        hx = hs_pool.tile([H, BT], F32, name="hx")
                        nc.scalar.activation(hx[:], hp[:], AF.Identity, bias=ct["Bse1"][:])
                        nc.vector.tensor_tensor(hs[:], sg[:], hx[:], mybir.AluOpType.mult)
                    else:
                        nc.scalar.activation(hs[:], hp[:], AF.Silu, bias=ct["Bse1"][:])

                    ap = ap_pool.tile([H, 2, BT], F32, name="ap", tag="ap")
                    nc.tensor.matmul(ap[:, 0, :], ct["LT_At"][:], hs[:], start=True, stop=True)
                    nc.tensor.matmul(ap[:, 1, :], ct["LT_Ab"][:], hs[:], start=True, stop=True)
                    acat = acat_pool.tile([H, 2, BT], BF16, name="acat")
                    if zero_bias:
                        nc.scalar.activation(acat[:], ap[:], AF.Identity)
                    else:
                        nc.scalar.activation(acat[:, 0, :], ap[:, 0, :], AF.Identity,
                                             bias=ct["Bse2t"][:])
                        nc.scalar.activation(acat[:, 1, :], ap[:, 1, :], AF.Identity,
                                             bias=ct["Bse2b"][:])
                    acats[j] = acat

                    ycat = ycat_pool.tile([H, 2, BT], BF16, name="ycat0")
                    eng = nc.gpsimd if YCAT0_ON_GPSIMD else nc.vector
                    eng.tensor_tensor(
                        ycat[:], acat[:],
                        zv4[:, j, :][:, None, :].broadcast_to([H, 2, BT]),
                        mybir.AluOpType.mult,
                    )
                    ycats[j] = ycat

                # ---- Horner steps k=4,3,2 ----
                for lt_top, lt_bot in taylor:
                    pvs = {}
                    for jp in ((0, 1), (2, 3)):
                        for j in jp:  # row-packed pair of +z folds
                            pv = pv_pool.tile([H, BT], F32, name="pv", tag="pv")
                            nc.tensor.matmul(
                                pv[:], ct["LT_z4"][bass.ts(j, 32), :], zv4[bass.ts(j, 32), j, :],
                                start=True, stop=False, skip_group_check=True,
                                tile_position=(32 * j, 0),
                            )
                            pvs[j] = pv
                        for j in jp:
                            nc.tensor.matmul(pvs[j][:], lt_top[:], ycats[j][:, 0, :],
                                             start=False, stop=False, skip_group_check=True)
                            nc.tensor.matmul(pvs[j][:], lt_bot[:], ycats[j][:, 1, :],
                                             start=False, stop=True, skip_group_check=True)
                        for j in jp:
                            ycat = ycat_pool.tile([H, 2, BT], BF16, name="ycat")
                            nc.vector.tensor_tensor(
                                ycat[:], acats[j][:],
                                pvs[j][:, None, :].broadcast_to([H, 2, BT]),
                                mybir.AluOpType.mult,
                            )
                            ycats[j] = ycat

                # ---- k=1 step fused with W1, then norm-sq ----
                sqs = {}
                for jp in ((0, 1), (2, 3)):
                    h1ps = {}
                    for j in jp:
                        h1p = pv_pool.tile([H, BT], F32, name="h1p", tag="pv")
                        nc.tensor.matmul(
                            h1p[:], ct["LT_w1z4"][bass.ts(j, 32), :], zv4[bass.ts(j, 32), j, :],
                            start=True, stop=False, skip_group_check=True,
                            tile_position=(32 * j, 0),
                        )
                        h1ps[j] = h1p
                    for j in jp:
                        nc.tensor.matmul(h1ps[j][:], ct["LT_t1"][:], ycats[j][:, 0, :],
                                         start=False, stop=False, skip_group_check=True)
                        nc.tensor.matmul(h1ps[j][:], ct["LT_b1"][:], ycats[j][:, 1, :],
                                         start=False, stop=True, skip_group_check=True)
                    for j in jp:
                        h1s = h1s_pool.tile([H, BT], BF16, name="h1s")
                        nc.scalar.activation(h1s[:], h1ps[j][:], AF.Identity, bias=ct["B1"][:])
                        h1s_tiles.append(h1s)
                        sq = sq_pool.tile([H, BT], BF16, name="sq")
                        nc.scalar.activation(sq[:], h1ps[j][:], AF.Square, bias=ct["B1"][:])
                        sqs[j] = sq

                # norm-sq of tile 4g+j -> col-group j, row g (4 col-packed MMs)
                for j in range(GS):
                    nc.tensor.matmul(
                        nsq_ps[bass.ts(j, 32), :], ct["onsq4"][:, bass.ts(g, 32)], sqs[j][:],
                        start=(g == 0), stop=(g == ngrp - 1), skip_group_check=True,
                        tile_position=(0, 32 * j),
                    )

            # ============== gate (batched sqrt + tanh) ==============
            rt = gate_pool.tile([H, BT], F32, name="rt")
            nc.scalar.activation(rt[:], nsq_ps[:], AF.Sqrt, bias=zero_b[:])
            t_all = gate_pool.tile([H, BT], BF16, name="t_all")
            # sigmoid(norm + eps) = 0.5 tanh(0.5 norm + eps/2) + 0.5
            nc.scalar.activation(t_all[:], rt[:], AF.Tanh, scale=0.5, bias=tanh_b[:])

            # ================= phase B =================
            for g in range(ngrp):
                trp = pso_pool.tile([H, BT], F32, name="trp", tag="pso")
                nc.tensor.matmul(trp[:], ct["E4"][:, bass.ts(g, H)], t_all[:],
                                 start=True, stop=True)
                t2g = gate_pool.tile([H, BT], BF16, name="t2g")
                nc.scalar.activation(t2g[:], trp[:], AF.Identity, bias=half_b[:])

                wp = pv_pool.tile([H, BT], F32, name="wp", tag="pv")
                for j in range(GS):  # col-packed W2 h1, M=32 each
                    nc.tensor.matmul(
                        wp[bass.ts(j, 32), :], ct["LT_w2c"][:, bass.ts(j, 32)],
                        h1s_tiles[GS * g + j][:],
                        start=True, stop=True, skip_group_check=True,
                        tile_position=(0, 32 * j),
                    )
                og = og_pool.tile([H, BT], F32, name="og")
                nc.vector.tensor_tensor(og[:], wp[:], t2g[:], mybir.AluOpType.mult)
                if not zero_bias:
                    og2 = og_pool.tile([H, BT], F32, name="og2")
                    nc.scalar.activation(og2[:], og[:], AF.Identity, bias=ct["B2r"][:])
                    og = og2
                for s in range(4):
                    nc.sync.dma_start(
                        outT[:, bass.ts(GS * g + s, BT)], og[bass.ts(s, 32), :]
                    )

    if split_waits:
        _split_multi_waits(nc)
    return nc


def _host_params(G, W_se1, b_se1, W_se2, b_se2, W1, b1, W2, b2, nt):
    import ml_dtypes
    f = np.float32
    bf = ml_dtypes.bfloat16
    ngrp = nt // GS
    G = np.asarray(G, f)
    Gflat = np.transpose(G, (0, 2, 1)).reshape(NG * D, D)  # [(g,i), j] = G[g,j,i]
    W1G = Gflat @ np.asarray(W1, f).T                      # [(g,i), m]
    I32 = np.eye(D, dtype=f)

    onsq = np.zeros((H, 4, 32), f)
    for r in range(4):
        onsq[:, r, r] = 1.0
    E4 = np.zeros((H, ngrp, H), f)
    for g in range(ngrp):
        for r in range(GS):
            E4[32 * r + g, g, 32 * r:32 * (r + 1)] = 0.5

    p = {
        "LT_h4": np.tile(np.asarray(W_se1, f).T, (4, 1)),
        "LT_At": np.repeat(np.asarray(W_se2, f).T[:, 0:4], 32, axis=1),
        "LT_Ab": np.repeat(np.asarray(W_se2, f).T[:, 4:8], 32, axis=1),
        "Bse1": np.asarray(b_se1, f).reshape(H, 1),
        "Bse2t": np.repeat(np.asarray(b_se2, f)[0:4], 32).reshape(H, 1),
        "Bse2b": np.repeat(np.asarray(b_se2, f)[4:8], 32).reshape(H, 1),
        "LT_z4": np.tile(I32, (4, 4)),
        "LT_w1z4": np.tile(np.asarray(W1, f).T, (4, 1)),
        "B1": np.asarray(b1, f).reshape(H, 1),
        "LT_t1": np.ascontiguousarray(W1G[:H]),
        "LT_b1": np.ascontiguousarray(W1G[H:]),
        "onsq4": onsq.reshape(H, H),
        "E4": E4.reshape(H, ngrp * H),
        "LT_w2c": np.tile(np.asarray(W2, f).T, (1, 4)),
        "B2r": np.tile(np.asarray(b2, f), 4).reshape(H, 1),
    }
    for k, tname, bname in ((4, "LT_t4", "LT_b4"), (3, "LT_t3", "LT_b3"), (2, "LT_t2", "LT_b2")):
        scaled = np.tile(Gflat * f(1.0 / k), (1, 4))
        p[tname] = np.ascontiguousarray(scaled[:H])
        p[bname] = np.ascontiguousarray(scaled[H:])
    for name, (shape, dt) in _PARAM_SHAPES.items():
        assert list(p[name].shape) == shape, (name, p[name].shape, shape)
        if dt == BF16:
            p[name] = p[name].astype(bf)
        else:
            p[name] = np.ascontiguousarray(p[name], f)
    return p


def _run(z, G, W_se1, b_se1, W_se2, b_se2, W1, b1, W2, b2, trace=False, **trace_kw):
    import ml_dtypes
    z = np.asarray(z, np.float32)
    nt = BC // BT
    # b_se1/b1 go through ACT bias slots either way; only b_se2/b2 change
    # the instruction count.
    zero_bias = (float(np.abs(np.asarray(b_se2)).max()) == 0.0
                 and float(np.abs(np.asarray(b2)).max()) == 0.0)
    params = _host_params(G, W_se1, b_se1, W_se2, b_se2, W1, b1, W2, b2, nt)

    zT = np.ascontiguousarray(
        z.reshape(NCORES, BC, D).transpose(0, 2, 1)
    ).astype(ml_dtypes.bfloat16)

    nc = _build_program(BC, zero_bias)
    in_maps = [{"zT": zT[c], **params} for c in range(NCORES)]
    res = run_bass_kernel_spmd(nc, in_maps, list(range(NCORES)), trace=trace, **trace_kw)

    outT = np.stack([res.results[c]["outT"] for c in range(NCORES)])
    out = outT.transpose(0, 2, 1).reshape(B, D)
    return np.ascontiguousarray(out.astype(np.float32)), res


def kernel(z, G, W_se1, b_se1, W_se2, b_se2, W1, b1, W2, b2):
    out, _ = _run(z, G, W_se1, b_se1, W_se2, b_se2, W1, b1, W2, b2, trace=False)
    return out


if __name__ == "__main__":
    rng = np.random.default_rng(0)
    inputs = {
        "z": rng.standard_normal((B, D), dtype=np.float32),
        "G": (rng.standard_normal((NG, D, D)) * 0.1).astype(np.float32),
        "W_se1": (rng.standard_normal((H, D)) / np.sqrt(D)).astype(np.float32),
        "b_se1": np.zeros(H, np.float32),
        "W_se2": (rng.standard_normal((NG, H)) / np.sqrt(H)).astype(np.float32),
        "b_se2": np.zeros(NG, np.float32),
        "W1": (rng.standard_normal((H, D)) * 0.01).astype(np.float32),
        "b1": np.zeros(H, np.float32),
        "W2": (rng.standard_normal((D, H)) * 0.01).astype(np.float32),
        "b2": np.zeros(D, np.float32),
    }
    out = kernel(**inputs)
    print("kernel output", out.shape, out.dtype, float(np.abs(out).max()))


# revision 12
# speedup vs baseline: 1.9248x; 1.4523x over previous
"""EquivariantEvolution kernel for 8 Trainium2 NeuronCores (Bass/Tile).

Math (per sample):
    alpha = W_se2 silu(W_se1 z + b_se1) + b_se2            # [NG=8]
    A     = sum_g alpha_g G_g                              # [32, 32]
    z_t   = (I + A + A^2/2 + A^3/6 + A^4/24) z             # order-4 Taylor
    h1    = W1 z_t + b1                                    # [128]
    out   = sigmoid(|h1| + eps) * (W2 h1) + b2             # gate commuted past W2

Device strategy (pure batch data-parallel, feature-major [feat, samples]):
  * Horner: v <- z + (1/k) A v.  A v as y[(g,i),b] = alpha_g[b] v_i[b]
    (elementwise outer product) contracted by two K=128 matmuls whose
    lhsT is pre-replicated 4x along M so outputs land ready for the next
    elementwise step.  The +z fold is a K=32 matmul row-packed 2x via
    tile_position so pairs run concurrently in the PE array.
  * Everything the PE touches is bf16 (FWL weight loads, half DMA);
    all accumulation stays f32 in PSUM.
  * The norm-squared reduction for all 16 tiles accumulates into ONE
    PSUM bank (4 col-groups x 4 rows), so the sqrt/tanh run once per
    kernel (2 ACT table switches total).
  * Phase B: gate = 0.5*tanh+0.5 is broadcast to 4 tiles at once by a
    single 0/0.5 matmul; W2 h1 is col-packed 4x (M=32 each); one DVE
    multiply + one 4-strip DMA store finishes 4 tiles.
  * No HAM warm-up spam: a short burst of zero matmuls during the
    initial parameter DMAs brings the clock to K=8/8; after that the
    real matmul stream is dense enough to keep it there.
"""

import os
import sys

import numpy as np

for _p in ("/opt/trn_rl_repo", "/root/.axon_site/_ro/trn_rl_repo"):
    if os.path.isdir(_p) and _p not in sys.path:
        sys.path.insert(0, _p)

import concourse.bass as bass
import concourse.mybir as mybir
import concourse.tile as tile
from concourse.bass_utils import run_bass_kernel_spmd

B, D, H, NG = 65536, 32, 128, 8
NCORES = 8
BC = B // NCORES          # samples per core
BT = 512                  # samples per tile (PSUM bank width in f32)
GS = 4                    # tiles per group
EPS = 1e-6
F32 = mybir.dt.float32
F32R = mybir.dt.float32r
BF16 = mybir.dt.bfloat16
AF = mybir.ActivationFunctionType

# whether the z-path outer product (all-SBUF operands) runs on gpsimd
YCAT0_ON_GPSIMD = True


def _split_multi_waits(nc, max_waits=1):
    """This toolchain's walrus rejects >1 sync-wait on an instruction
    ("Too many sync wait commands"); hoist extra waits onto preceding
    same-engine NOPs (in-order engines make this semantics-preserving)."""
    n_new = 0
    for f in nc.m.functions:
        for bb in f.blocks:
            out = []
            for ins in bb.instructions:
                si = getattr(ins, "sync_info", None)
                if si is not None and si.on_wait and len(si.on_wait) > max_waits:
                    waits = list(si.on_wait)
                    chunks = [waits[i:i + max_waits] for i in range(0, len(waits), max_waits)]
                    for ci, ch in enumerate(chunks[:-1]):
                        nop = mybir.InstNoOp(
                            name=f"{ins.name}-wsplit{ci}",
                            engine=ins.engine,
                            sync_info=mybir.SyncInfo(on_wait=ch, on_update=[]),
                            bass_nofuse=True,
                        )
                        out.append(nop)
                        n_new += 1
                    ins.sync_info = mybir.SyncInfo(on_wait=chunks[-1], on_update=si.on_update)
                out.append(ins)
            bb.instructions[:] = out
    return n_new


# DRAM parameters: name -> (shape, dtype).  All matmul operands bf16.
_PARAM_SHAPES = {
    "LT_h4": ([H, H], BF16),      # W_se1^T tiled 4x along partitions
    "LT_At": ([H, H], BF16),      # W_se2[0:4] replicated 32x over M
    "LT_Ab": ([H, H], BF16),      # W_se2[4:8]
    "Bse1": ([H, 1], F32),
    "Bse2t": ([H, 1], F32),
    "Bse2b": ([H, 1], F32),
    "LT_t4": ([H, H], BF16), "LT_b4": ([H, H], BF16),
    "LT_t3": ([H, H], BF16), "LT_b3": ([H, H], BF16),
    "LT_t2": ([H, H], BF16), "LT_b2": ([H, H], BF16),
    "LT_t1": ([H, H], BF16), "LT_b1": ([H, H], BF16),   # W1-folded k=1 step
    "LT_z4": ([H, H], BF16),      # I32 tiled (4,4): row-packable +z fold
    "LT_w1z4": ([H, H], BF16),    # W1^T tiled 4x along partitions
    "B1": ([H, 1], F32),
    "onsq4": ([H, H], BF16),      # norm-sq row-select weights (variant r at cols 32r)
    "E4": ([H, 4 * H], BF16),     # 0.5-scaled gate broadcast, one [H,H] block per group
    "LT_w2c": ([H, H], BF16),     # W2^T tiled 4x along M (col-packable)
    "B2r": ([H, 1], F32),         # b2 tiled 4x along partitions
}


def _build_program(bc: int, zero_bias: bool, sim_safe: bool = False, split_waits: bool = True):
    nt = bc // BT
    ngrp = nt // GS
    nc = bass.Bass()

    zT = nc.declare_dram_parameter("zT", [D, bc], BF16, isOutput=False)
    params = {
        name: nc.declare_dram_parameter(name, shape, dt, isOutput=False)
        for name, (shape, dt) in _PARAM_SHAPES.items()
    }
    outT = nc.declare_dram_parameter("outT", [D, bc], F32, isOutput=True)

    with tile.TileContext(nc) as tc:
        with (
            tc.tile_pool(name="consts", bufs=1) as consts,
            tc.tile_pool(name="zv4", bufs=2) as zv4_pool,
            tc.tile_pool(name="hs", bufs=3) as hs_pool,
            tc.tile_pool(name="acat", bufs=5) as acat_pool,   # [H,2,2,BT] pair tiles
            tc.tile_pool(name="ycat", bufs=5) as ycat_pool,   # [H,2,2,BT] pair tiles
            tc.tile_pool(name="sq", bufs=5) as sq_pool,
            tc.tile_pool(name="h1s", bufs=nt) as h1s_pool,
            tc.tile_pool(name="gate", bufs=4) as gate_pool,
            tc.tile_pool(name="og", bufs=3) as og_pool,
            # PSUM: hp(1) + ap(2) + pv(2x2) + nsq(1) = 8 banks
            tc.tile_pool(name="hp", bufs=1, space=bass.MemorySpace.PSUM) as hp_pool,
            tc.tile_pool(name="ap", bufs=1, space=bass.MemorySpace.PSUM) as ap_pool,
            tc.tile_pool(name="pv", bufs=2, space=bass.MemorySpace.PSUM) as pv_pool,
            tc.tile_pool(name="psn", bufs=1, space=bass.MemorySpace.PSUM) as psn_pool,
        ):
            # ---- constants into SBUF ----
            ct = {}
            for name, (shape, dt) in _PARAM_SHAPES.items():
                t = consts.tile(shape, dt, name=f"c_{name}")
                nc.sync.dma_start(t[:], params[name][:])
                ct[name] = t
            half_b = consts.tile([H, 1], F32, name="half_b")
            nc.vector.memset(half_b[:], 0.5)
            zero_b = consts.tile([H, 1], F32, name="zero_b")
            nc.vector.memset(zero_b[:], 0.0)
            tanh_b = consts.tile([H, 1], F32, name="tanh_b")
            nc.vector.memset(tanh_b[:], 0.5 * EPS)

            # ---- HAM ramp: zero-matmuls run while parameter DMAs land ----
            wscr = consts.tile([H, BT], BF16, name="wscr")
            nc.vector.memset(wscr[:], 0.0)
            wps = psn_pool.tile([H, BT], F32, name="wps", tag="nsq")
            for _ in range(12):
                nc.tensor.matmul(wps[:], wscr[:, 0:H], wscr[:], start=True, stop=True)

            taylor = [
                (ct["LT_t4"], ct["LT_b4"]),
                (ct["LT_t3"], ct["LT_b3"]),
                (ct["LT_t2"], ct["LT_b2"]),
            ]

            nsq_ps = psn_pool.tile([H, BT], F32, name="nsq_ps", tag="nsq")
            h1s_tiles = []
            PAIRS = (0, 2)

            def emit_loads(g):
                zv4 = zv4_pool.tile([H, GS, BT], BF16, name="zv4")
                for s in range(4):
                    nc.gpsimd.dma_start(
                        zv4[32 * s:32 * (s + 1), :, :],
                        zT[:, bass.ts(g, GS * BT)],
                    )
                return zv4

            def emit_extractor(zv4):
                """alpha for 4 tiles; returns per-pair acat/ycat0 [H,2,2,BT]."""
                acats, ycats = {}, {}
                for jp in PAIRS:
                    acat = acat_pool.tile([H, 2, 2, BT], BF16, name="acat")
                    for jj in range(2):
                        j = jp + jj
                        hp = hp_pool.tile([H, BT], F32, name="hp", tag="hp")
                        nc.tensor.matmul(hp[:], ct["LT_h4"][0:D, :], zv4[0:D, j, :],
                                         start=True, stop=True)
                        hs = hs_pool.tile([H, BT], BF16, name="hs")
                        if sim_safe:
                            sg = hs_pool.tile([H, BT], F32, name="sg")
                            nc.scalar.activation(sg[:], hp[:], AF.Sigmoid, bias=ct["Bse1"][:])
                            hx = hs_pool.tile([H, BT], F32, name="hx")
                            nc.scalar.activation(hx[:], hp[:], AF.Identity, bias=ct["Bse1"][:])
                            nc.vector.tensor_tensor(hs[:], sg[:], hx[:], mybir.AluOpType.mult)
                        else:
                            nc.scalar.activation(hs[:], hp[:], AF.Silu, bias=ct["Bse1"][:])

                        ap = ap_pool.tile([H, 2, BT], F32, name="ap", tag="ap")
                        nc.tensor.matmul(ap[:, 0, :], ct["LT_At"][:], hs[:],
                                         start=True, stop=True)
                        nc.tensor.matmul(ap[:, 1, :], ct["LT_Ab"][:], hs[:],
                                         start=True, stop=True)
                        if zero_bias:
                            nc.scalar.activation(acat[:, :, jj, :], ap[:], AF.Identity)
                        else:
                            nc.scalar.activation(acat[:, 0, jj, :], ap[:, 0, :], AF.Identity,
                                                 bias=ct["Bse2t"][:])
                            nc.scalar.activation(acat[:, 1, jj, :], ap[:, 1, :], AF.Identity,
                                                 bias=ct["Bse2b"][:])
                    acats[jp] = acat

                    ycat = ycat_pool.tile([H, 2, 2, BT], BF16, name="ycat0")
                    eng = nc.gpsimd if YCAT0_ON_GPSIMD else nc.vector
                    eng.tensor_tensor(
                        ycat[:], acat[:],
                        zv4[:, None, jp:jp + 2, :].broadcast_to([H, 2, 2, BT]),
                        mybir.AluOpType.mult,
                    )
                    ycats[jp] = ycat
                return acats, ycats

            def emit_taylor(zv4, acats, ycats):
                """Horner k=4,3,2 then W1-fused k=1; h1s/sq per tile."""
                for lt_top, lt_bot in taylor:
                    for jp in PAIRS:
                        pv2 = pv_pool.tile([H, 2, BT], F32, name="pv2", tag="pv")
                        for jj in range(2):
                            j = jp + jj
                            nc.tensor.matmul(
                                pv2[:, jj, :], ct["LT_z4"][bass.ts(j, 32), :],
                                zv4[bass.ts(j, 32), j, :],
                                start=True, stop=False, skip_group_check=True,
                                tile_position=(32 * j, 0),
                            )
                        for jj in range(2):
                            nc.tensor.matmul(pv2[:, jj, :], lt_top[:],
                                             ycats[jp][:, 0, jj, :],
                                             start=False, stop=False, skip_group_check=True)
                            nc.tensor.matmul(pv2[:, jj, :], lt_bot[:],
                                             ycats[jp][:, 1, jj, :],
                                             start=False, stop=True, skip_group_check=True)
                        ycat = ycat_pool.tile([H, 2, 2, BT], BF16, name="ycat")
                        nc.vector.tensor_tensor(
                            ycat[:], acats[jp][:],
                            pv2[:, None, :, :].broadcast_to([H, 2, 2, BT]),
                            mybir.AluOpType.mult,
                        )
                        ycats[jp] = ycat

                sqs = {}
                for jp in PAIRS:
                    h1p2 = pv_pool.tile([H, 2, BT], F32, name="h1p2", tag="pv")
                    for jj in range(2):
                        j = jp + jj
                        nc.tensor.matmul(
                            h1p2[:, jj, :], ct["LT_w1z4"][bass.ts(j, 32), :],
                            zv4[bass.ts(j, 32), j, :],
                            start=True, stop=False, skip_group_check=True,
                            tile_position=(32 * j, 0),
                        )
                    for jj in range(2):
                        nc.tensor.matmul(h1p2[:, jj, :], ct["LT_t1"][:],
                                         ycats[jp][:, 0, jj, :],
                                         start=False, stop=False, skip_group_check=True)
                        nc.tensor.matmul(h1p2[:, jj, :], ct["LT_b1"][:],
                                         ycats[jp][:, 1, jj, :],
                                         start=False, stop=True, skip_group_check=True)
                    for jj in range(2):
                        j = jp + jj
                        h1s = h1s_pool.tile([H, BT], BF16, name="h1s")
                        nc.scalar.activation(h1s[:], h1p2[:, jj, :], AF.Identity,
                                             bias=ct["B1"][:])
                        h1s_tiles.append(h1s)
                        sq = sq_pool.tile([H, BT], BF16, name="sq")
                        nc.scalar.activation(sq[:], h1p2[:, jj, :], AF.Square,
                                             bias=ct["B1"][:])
                        sqs[j] = sq
                return sqs

            def emit_nsq(g, sqs):
                # norm-sq of tile 4g+j -> col-group j, row g (4 col-packed MMs)
                for j in range(GS):
                    nc.tensor.matmul(
                        nsq_ps[bass.ts(j, 32), :], ct["onsq4"][:, bass.ts(g, 32)], sqs[j][:],
                        start=(g == 0), stop=(g == ngrp - 1), skip_group_check=True,
                        tile_position=(0, 32 * j),
                    )

            # ================= phase A (software-pipelined groups) =================
            zv4_g = emit_loads(0)
            ext_g = emit_extractor(zv4_g)
            state = (zv4_g,) + ext_g
            for g in range(ngrp):
                zv4_g, acats_g, ycats_g = state
                if g + 1 < ngrp:
                    zv4_n = emit_loads(g + 1)
                    ext_n = emit_extractor(zv4_n)
                    state = (zv4_n,) + ext_n
                sqs = emit_taylor(zv4_g, acats_g, ycats_g)
                emit_nsq(g, sqs)

            # ============== gate (batched sqrt + tanh) ==============
            rt = gate_pool.tile([H, BT], F32, name="rt")
            nc.scalar.activation(rt[:], nsq_ps[:], AF.Sqrt, bias=zero_b[:])
            t_all = gate_pool.tile([H, BT], BF16, name="t_all")
            # sigmoid(norm + eps) = 0.5 tanh(0.5 norm + eps/2) + 0.5
            nc.scalar.activation(t_all[:], rt[:], AF.Tanh, scale=0.5, bias=tanh_b[:])

            # ================= phase B =================
            for g in range(ngrp):
                trp = hp_pool.tile([H, BT], F32, name="trp", tag="hp")
                nc.tensor.matmul(trp[:], ct["E4"][:, bass.ts(g, H)], t_all[:],
                                 start=True, stop=True)
                t2g = gate_pool.tile([H, BT], BF16, name="t2g")
                nc.scalar.activation(t2g[:], trp[:], AF.Identity, bias=half_b[:])

                wp = pv_pool.tile([H, BT], F32, name="wp", tag="pv")
                for j in range(GS):  # col-packed W2 h1, M=32 each
                    nc.tensor.matmul(
                        wp[bass.ts(j, 32), :], ct["LT_w2c"][:, bass.ts(j, 32)],
                        h1s_tiles[GS * g + j][:],
                        start=True, stop=True, skip_group_check=True,
                        tile_position=(0, 32 * j),
                    )
                og = og_pool.tile([H, BT], F32, name="og")
                nc.vector.tensor_tensor(og[:], wp[:], t2g[:], mybir.AluOpType.mult)
                if not zero_bias:
                    og2 = og_pool.tile([H, BT], F32, name="og2")
                    nc.scalar.activation(og2[:], og[:], AF.Identity, bias=ct["B2r"][:])
                    og = og2
                for s in range(4):
                    nc.sync.dma_start(
                        outT[:, bass.ts(GS * g + s, BT)], og[bass.ts(s, 32), :]
                    )

    if split_waits:
        _split_multi_waits(nc)
    return nc


def _host_params(G, W_se1, b_se1, W_se2, b_se2, W1, b1, W2, b2, nt):
    import ml_dtypes
    f = np.float32
    bf = ml_dtypes.bfloat16
    ngrp = nt // GS
    G = np.asarray(G, f)
    Gflat = np.transpose(G, (0, 2, 1)).reshape(NG * D, D)  # [(g,i), j] = G[g,j,i]
    W1G = Gflat @ np.asarray(W1, f).T                      # [(g,i), m]
    I32 = np.eye(D, dtype=f)

    onsq = np.zeros((H, 4, 32), f)
    for r in range(4):
        onsq[:, r, r] = 1.0
    E4 = np.zeros((H, ngrp, H), f)
    for g in range(ngrp):
        for r in range(GS):
            E4[32 * r + g, g, 32 * r:32 * (r + 1)] = 0.5

    p = {
        "LT_h4": np.tile(np.asarray(W_se1, f).T, (4, 1)),
        "LT_At": np.repeat(np.asarray(W_se2, f).T[:, 0:4], 32, axis=1),
        "LT_Ab": np.repeat(np.asarray(W_se2, f).T[:, 4:8], 32, axis=1),
        "Bse1": np.asarray(b_se1, f).reshape(H, 1),
        "Bse2t": np.repeat(np.asarray(b_se2, f)[0:4], 32).reshape(H, 1),
        "Bse2b": np.repeat(np.asarray(b_se2, f)[4:8], 32).reshape(H, 1),
        "LT_z4": np.tile(I32, (4, 4)),
        "LT_w1z4": np.tile(np.asarray(W1, f).T, (4, 1)),
        "B1": np.asarray(b1, f).reshape(H, 1),
        "LT_t1": np.ascontiguousarray(W1G[:H]),
        "LT_b1": np.ascontiguousarray(W1G[H:]),
        "onsq4": onsq.reshape(H, H),
        "E4": E4.reshape(H, ngrp * H),
        "LT_w2c": np.tile(np.asarray(W2, f).T, (1, 4)),
        "B2r": np.tile(np.asarray(b2, f), 4).reshape(H, 1),
    }
    for k, tname, bname in ((4, "LT_t4", "LT_b4"), (3, "LT_t3", "LT_b3"), (2, "LT_t2", "LT_b2")):
        scaled = np.tile(Gflat * f(1.0 / k), (1, 4))
        p[tname] = np.ascontiguousarray(scaled[:H])
        p[bname] = np.ascontiguousarray(scaled[H:])
    for name, (shape, dt) in _PARAM_SHAPES.items():
        assert list(p[name].shape) == shape, (name, p[name].shape, shape)
        if dt == BF16:
            p[name] = p[name].astype(bf)
        else:
            p[name] = np.ascontiguousarray(p[name], f)
    return p


def _run(z, G, W_se1, b_se1, W_se2, b_se2, W1, b1, W2, b2, trace=False, **trace_kw):
    import ml_dtypes
    z = np.asarray(z, np.float32)
    nt = BC // BT
    # b_se1/b1 go through ACT bias slots either way; only b_se2/b2 change
    # the instruction count.
    zero_bias = (float(np.abs(np.asarray(b_se2)).max()) == 0.0
                 and float(np.abs(np.asarray(b2)).max()) == 0.0)
    params = _host_params(G, W_se1, b_se1, W_se2, b_se2, W1, b1, W2, b2, nt)

    zT = np.ascontiguousarray(
        z.reshape(NCORES, BC, D).transpose(0, 2, 1)
    ).astype(ml_dtypes.bfloat16)

    nc = _build_program(BC, zero_bias)
    in_maps = [{"zT": zT[c], **params} for c in range(NCORES)]
    res = run_bass_kernel_spmd(nc, in_maps, list(range(NCORES)), trace=trace, **trace_kw)

    outT = np.stack([res.results[c]["outT"] for c in range(NCORES)])
    out = outT.transpose(0, 2, 1).reshape(B, D)
    return np.ascontiguousarray(out.astype(np.float32)), res


def kernel(z, G, W_se1, b_se1, W_se2, b_se2, W1, b1, W2, b2):
    out, _ = _run(z, G, W_se1, b_se1, W_se2, b_se2, W1, b1, W2, b2, trace=False)
    return out


if __name__ == "__main__":
    rng = np.random.default_rng(0)
    inputs = {
        "z": rng.standard_normal((B, D), dtype=np.float32),
        "G": (rng.standard_normal((NG, D, D)) * 0.1).astype(np.float32),
        "W_se1": (rng.standard_normal((H, D)) / np.sqrt(D)).astype(np.float32),
        "b_se1": np.zeros(H, np.float32),
        "W_se2": (rng.standard_normal((NG, H)) / np.sqrt(H)).astype(np.float32),
        "b_se2": np.zeros(NG, np.float32),
        "W1": (rng.standard_normal((H, D)) * 0.01).astype(np.float32),
        "b1": np.zeros(H, np.float32),
        "W2": (rng.standard_normal((D, H)) * 0.01).astype(np.float32),
        "b2": np.zeros(D, np.float32),
    }
    out = kernel(**inputs)
    print("kernel output", out.shape, out.dtype, float(np.abs(out).max()))


# revision 16
# speedup vs baseline: 1.9754x; 1.0263x over previous
"""EquivariantEvolution kernel for 8 Trainium2 NeuronCores (Bass/Tile).

Math (per sample):
    alpha = W_se2 silu(W_se1 z + b_se1) + b_se2            # [NG=8]
    A     = sum_g alpha_g G_g                              # [32, 32]
    z_t   = (I + A + A^2/2 + A^3/6 + A^4/24) z             # order-4 Taylor
    h1    = W1 z_t + b1                                    # [128]
    out   = sigmoid(|h1| + eps) * (W2 h1) + b2             # gate commuted past W2

Device strategy (pure batch data-parallel, feature-major [feat, samples]):
  * Horner: v <- z + (1/k) A v.  A v as y[(g,i),b] = alpha_g[b] v_i[b]
    (elementwise outer product) contracted by two K=128 matmuls whose
    lhsT is pre-replicated 4x along M so outputs land ready for the next
    elementwise step.  The +z fold is a K=32 matmul row-packed 2x via
    tile_position so pairs run concurrently in the PE array.
  * Everything the PE touches is bf16 (FWL weight loads, half DMA);
    all accumulation stays f32 in PSUM.
  * The norm-squared reduction for all 16 tiles accumulates into ONE
    PSUM bank (4 col-groups x 4 rows), so the sqrt/tanh run once per
    kernel (2 ACT table switches total).
  * Phase B: gate = 0.5*tanh+0.5 is broadcast to 4 tiles at once by a
    single 0/0.5 matmul; W2 h1 is col-packed 4x (M=32 each); one DVE
    multiply + one 4-strip DMA store finishes 4 tiles.
  * No HAM warm-up spam: a short burst of zero matmuls during the
    initial parameter DMAs brings the clock to K=8/8; after that the
    real matmul stream is dense enough to keep it there.
"""

import os
import sys

import numpy as np

for _p in ("/opt/trn_rl_repo", "/root/.axon_site/_ro/trn_rl_repo"):
    if os.path.isdir(_p) and _p not in sys.path:
        sys.path.insert(0, _p)

import concourse.bass as bass
import concourse.mybir as mybir
import concourse.tile as tile
from concourse.bass_utils import run_bass_kernel_spmd

B, D, H, NG = 65536, 32, 128, 8
NCORES = 8
BC = B // NCORES          # samples per core
BT = 512                  # samples per tile (PSUM bank width in f32)
GS = 4                    # tiles per group
EPS = 1e-6
F32 = mybir.dt.float32
F32R = mybir.dt.float32r
BF16 = mybir.dt.bfloat16
AF = mybir.ActivationFunctionType

# whether the z-path outer product (all-SBUF operands) runs on gpsimd
YCAT0_ON_GPSIMD = True


def _split_multi_waits(nc, max_waits=1):
    """This toolchain's walrus rejects >1 sync-wait on an instruction
    ("Too many sync wait commands"); hoist extra waits onto preceding
    same-engine NOPs (in-order engines make this semantics-preserving)."""
    n_new = 0
    for f in nc.m.functions:
        for bb in f.blocks:
            out = []
            for ins in bb.instructions:
                si = getattr(ins, "sync_info", None)
                if si is not None and si.on_wait and len(si.on_wait) > max_waits:
                    waits = list(si.on_wait)
                    chunks = [waits[i:i + max_waits] for i in range(0, len(waits), max_waits)]
                    for ci, ch in enumerate(chunks[:-1]):
                        nop = mybir.InstNoOp(
                            name=f"{ins.name}-wsplit{ci}",
                            engine=ins.engine,
                            sync_info=mybir.SyncInfo(on_wait=ch, on_update=[]),
                            bass_nofuse=True,
                        )
                        out.append(nop)
                        n_new += 1
                    ins.sync_info = mybir.SyncInfo(on_wait=chunks[-1], on_update=si.on_update)
                out.append(ins)
            bb.instructions[:] = out
    return n_new


# DRAM parameters: name -> (shape, dtype).  All matmul operands bf16.
_PARAM_SHAPES = {
    "LT_h4": ([H, H], BF16),      # W_se1^T tiled 4x along partitions
    "LT_At": ([H, H], BF16),      # W_se2[0:4] replicated 32x over M
    "LT_Ab": ([H, H], BF16),      # W_se2[4:8]
    "Bse1": ([H, 1], F32),
    "Bse2t": ([H, 1], F32),
    "Bse2b": ([H, 1], F32),
    "LT_t4": ([H, H], BF16), "LT_b4": ([H, H], BF16),
    "LT_t3": ([H, H], BF16), "LT_b3": ([H, H], BF16),
    "LT_t2": ([H, H], BF16), "LT_b2": ([H, H], BF16),
    "LT_t1": ([H, H], BF16), "LT_b1": ([H, H], BF16),   # W1-folded k=1 step
    "LT_z4": ([H, H], BF16),      # I32 tiled (4,4): row-packable +z fold
    "LT_w1z4": ([H, H], BF16),    # W1^T tiled 4x along partitions
    "B1": ([H, 1], F32),
    "onsq4": ([H, H], BF16),      # norm-sq row-select weights (variant r at cols 32r)
    "E4": ([H, 4 * H], BF16),     # 0.5-scaled gate broadcast, one [H,H] block per group
    "LT_w2c": ([H, H], BF16),     # W2^T tiled 4x along M (col-packable)
    "B2r": ([H, 1], F32),         # b2 tiled 4x along partitions
}


def _build_program(bc: int, zero_bias: bool, sim_safe: bool = False, split_waits: bool = True):
    nt = bc // BT
    ngrp = nt // GS
    nc = bass.Bass()

    zT = nc.declare_dram_parameter("zT", [D, bc], BF16, isOutput=False)
    params = {
        name: nc.declare_dram_parameter(name, shape, dt, isOutput=False)
        for name, (shape, dt) in _PARAM_SHAPES.items()
    }
    outT = nc.declare_dram_parameter("outT", [D, bc], F32, isOutput=True)

    with tile.TileContext(nc) as tc:
        with (
            tc.tile_pool(name="consts", bufs=1) as consts,
            tc.tile_pool(name="zv4", bufs=2) as zv4_pool,
            tc.tile_pool(name="hs", bufs=3) as hs_pool,
            tc.tile_pool(name="acat", bufs=5) as acat_pool,   # [H,2,2,BT] pair tiles
            tc.tile_pool(name="ycat", bufs=5) as ycat_pool,   # [H,2,2,BT] pair tiles
            tc.tile_pool(name="sq", bufs=5) as sq_pool,
            tc.tile_pool(name="h1s", bufs=nt) as h1s_pool,
            tc.tile_pool(name="gate", bufs=4) as gate_pool,
            tc.tile_pool(name="og", bufs=3) as og_pool,
            # PSUM: hp(1) + ap(2) + pv(2x2) + nsq(1) = 8 banks
            tc.tile_pool(name="hp", bufs=1, space=bass.MemorySpace.PSUM) as hp_pool,
            tc.tile_pool(name="ap", bufs=1, space=bass.MemorySpace.PSUM) as ap_pool,
            tc.tile_pool(name="pv", bufs=2, space=bass.MemorySpace.PSUM) as pv_pool,
            tc.tile_pool(name="psn", bufs=1, space=bass.MemorySpace.PSUM) as psn_pool,
        ):
            # ---- constants into SBUF (issue spread across queues) ----
            ct = {}
            qs = (nc.sync, nc.scalar, nc.sync, nc.gpsimd)
            for qi, (name, (shape, dt)) in enumerate(_PARAM_SHAPES.items()):
                t = consts.tile(shape, dt, name=f"c_{name}")
                qs[qi % len(qs)].dma_start(t[:], params[name][:])
                ct[name] = t
            half_b = consts.tile([H, 1], F32, name="half_b")
            nc.vector.memset(half_b[:], 0.5)
            zero_b = consts.tile([H, 1], F32, name="zero_b")
            nc.vector.memset(zero_b[:], 0.0)
            tanh_b = consts.tile([H, 1], F32, name="tanh_b")
            nc.vector.memset(tanh_b[:], 0.5 * EPS)

            # ---- HAM ramp: zero-matmuls run while parameter DMAs land ----
            wscr = consts.tile([H, BT], BF16, name="wscr")
            nc.vector.memset(wscr[:], 0.0)
            wps = psn_pool.tile([H, BT], F32, name="wps", tag="nsq")
            for _ in range(20):
                nc.tensor.matmul(wps[:], wscr[:, 0:H], wscr[:], start=True, stop=True)

            taylor = [
                (ct["LT_t4"], ct["LT_b4"]),
                (ct["LT_t3"], ct["LT_b3"]),
                (ct["LT_t2"], ct["LT_b2"]),
            ]

            nsq_ps = psn_pool.tile([H, BT], F32, name="nsq_ps", tag="nsq")
            h1s_tiles = []
            PAIRS = (0, 2)

            def emit_loads(g):
                zv4 = zv4_pool.tile([H, GS, BT], BF16, name="zv4")
                for s in range(4):
                    nc.gpsimd.dma_start(
                        zv4[32 * s:32 * (s + 1), :, :],
                        zT[:, bass.ts(g, GS * BT)],
                    )
                return zv4

            def emit_extractor(zv4):
                """alpha for 4 tiles; returns per-pair acat/ycat0 [H,2,2,BT]."""
                acats, ycats = {}, {}
                for jp in PAIRS:
                    acat = acat_pool.tile([H, 2, 2, BT], BF16, name="acat")
                    for jj in range(2):
                        j = jp + jj
                        hp = hp_pool.tile([H, BT], F32, name="hp", tag="hp")
                        nc.tensor.matmul(hp[:], ct["LT_h4"][0:D, :], zv4[0:D, j, :],
                                         start=True, stop=True)
                        hs = hs_pool.tile([H, BT], BF16, name="hs")
                        if sim_safe:
                            sg = hs_pool.tile([H, BT], F32, name="sg")
                            nc.scalar.activation(sg[:], hp[:], AF.Sigmoid, bias=ct["Bse1"][:])
                            hx = hs_pool.tile([H, BT], F32, name="hx")
                            nc.scalar.activation(hx[:], hp[:], AF.Identity, bias=ct["Bse1"][:])
                            nc.vector.tensor_tensor(hs[:], sg[:], hx[:], mybir.AluOpType.mult)
                        else:
                            nc.scalar.activation(hs[:], hp[:], AF.Silu, bias=ct["Bse1"][:])

                        ap = ap_pool.tile([H, 2, BT], F32, name="ap", tag="ap")
                        nc.tensor.matmul(ap[:, 0, :], ct["LT_At"][:], hs[:],
                                         start=True, stop=True)
                        nc.tensor.matmul(ap[:, 1, :], ct["LT_Ab"][:], hs[:],
                                         start=True, stop=True)
                        if zero_bias:
                            nc.scalar.activation(acat[:, :, jj, :], ap[:], AF.Identity)
                        else:
                            nc.scalar.activation(acat[:, 0, jj, :], ap[:, 0, :], AF.Identity,
                                                 bias=ct["Bse2t"][:])
                            nc.scalar.activation(acat[:, 1, jj, :], ap[:, 1, :], AF.Identity,
                                                 bias=ct["Bse2b"][:])
                    acats[jp] = acat

                    ycat = ycat_pool.tile([H, 2, 2, BT], BF16, name="ycat0")
                    eng = nc.gpsimd if YCAT0_ON_GPSIMD else nc.vector
                    eng.tensor_tensor(
                        ycat[:], acat[:],
                        zv4[:, None, jp:jp + 2, :].broadcast_to([H, 2, 2, BT]),
                        mybir.AluOpType.mult,
                    )
                    ycats[jp] = ycat
                return acats, ycats

            def emit_taylor(zv4, acats, ycats):
                """Horner k=4,3,2 then W1-fused k=1; h1s/sq per tile."""
                for lt_top, lt_bot in taylor:
                    for jp in PAIRS:
                        pv2 = pv_pool.tile([H, 2, BT], F32, name="pv2", tag="pv")
                        for jj in range(2):
                            j = jp + jj
                            nc.tensor.matmul(
                                pv2[:, jj, :], ct["LT_z4"][bass.ts(j, 32), :],
                                zv4[bass.ts(j, 32), j, :],
                                start=True, stop=False, skip_group_check=True,
                                tile_position=(32 * j, 0),
                            )
                        for jj in range(2):
                            nc.tensor.matmul(pv2[:, jj, :], lt_top[:],
                                             ycats[jp][:, 0, jj, :],
                                             start=False, stop=False, skip_group_check=True)
                            nc.tensor.matmul(pv2[:, jj, :], lt_bot[:],
                                             ycats[jp][:, 1, jj, :],
                                             start=False, stop=True, skip_group_check=True)
                        ycat = ycat_pool.tile([H, 2, 2, BT], BF16, name="ycat")
                        nc.vector.tensor_tensor(
                            ycat[:], acats[jp][:],
                            pv2[:, None, :, :].broadcast_to([H, 2, 2, BT]),
                            mybir.AluOpType.mult,
                        )
                        ycats[jp] = ycat

                sqs = {}
                for jp in PAIRS:
                    h1p2 = pv_pool.tile([H, 2, BT], F32, name="h1p2", tag="pv")
                    for jj in range(2):
                        j = jp + jj
                        nc.tensor.matmul(
                            h1p2[:, jj, :], ct["LT_w1z4"][bass.ts(j, 32), :],
                            zv4[bass.ts(j, 32), j, :],
                            start=True, stop=False, skip_group_check=True,
                            tile_position=(32 * j, 0),
                        )
                    for jj in range(2):
                        nc.tensor.matmul(h1p2[:, jj, :], ct["LT_t1"][:],
                                         ycats[jp][:, 0, jj, :],
                                         start=False, stop=False, skip_group_check=True)
                        nc.tensor.matmul(h1p2[:, jj, :], ct["LT_b1"][:],
                                         ycats[jp][:, 1, jj, :],
                                         start=False, stop=True, skip_group_check=True)
                    for jj in range(2):
                        j = jp + jj
                        h1s = h1s_pool.tile([H, BT], BF16, name="h1s")
                        nc.scalar.activation(h1s[:], h1p2[:, jj, :], AF.Identity,
                                             bias=ct["B1"][:])
                        h1s_tiles.append(h1s)
                        sq = sq_pool.tile([H, BT], BF16, name="sq")
                        nc.scalar.activation(sq[:], h1p2[:, jj, :], AF.Square,
                                             bias=ct["B1"][:])
                        sqs[j] = sq
                return sqs

            def emit_nsq(g, sqs):
                # norm-sq of tile 4g+j -> col-group j, row g (4 col-packed MMs)
                for j in range(GS):
                    nc.tensor.matmul(
                        nsq_ps[bass.ts(j, 32), :], ct["onsq4"][:, bass.ts(g, 32)], sqs[j][:],
                        start=(g == 0), stop=(g == ngrp - 1), skip_group_check=True,
                        tile_position=(0, 32 * j),
                    )

            # ================= phase A (software-pipelined groups) =================
            zv4_g = emit_loads(0)
            ext_g = emit_extractor(zv4_g)
            state = (zv4_g,) + ext_g
            for g in range(ngrp):
                zv4_g, acats_g, ycats_g = state
                if g + 1 < ngrp:
                    zv4_n = emit_loads(g + 1)
                    ext_n = emit_extractor(zv4_n)
                    state = (zv4_n,) + ext_n
                sqs = emit_taylor(zv4_g, acats_g, ycats_g)
                emit_nsq(g, sqs)

            # ============== gate (batched sqrt + tanh) ==============
            rt = gate_pool.tile([H, BT], F32, name="rt")
            nc.scalar.activation(rt[:], nsq_ps[:], AF.Sqrt, bias=zero_b[:])
            t_all = gate_pool.tile([H, BT], BF16, name="t_all")
            # sigmoid(norm + eps) = 0.5 tanh(0.5 norm + eps/2) + 0.5
            nc.scalar.activation(t_all[:], rt[:], AF.Tanh, scale=0.5, bias=tanh_b[:])

            # ================= phase B (pipelined groups) =================
            for g in range(ngrp):
                pb = pv_pool.tile([H, 2, BT], F32, name="pb", tag="pv")
                trp, wp = pb[:, 0, :], pb[:, 1, :]
                nc.tensor.matmul(trp, ct["E4"][:, bass.ts(g, H)], t_all[:],
                                 start=True, stop=True, skip_group_check=True)
                t2g = gate_pool.tile([H, BT], BF16, name="t2g")
                nc.scalar.activation(t2g[:], trp, AF.Identity, bias=half_b[:])

                for j in range(GS):  # col-packed W2 h1, M=32 each
                    nc.tensor.matmul(
                        wp[bass.ts(j, 32), :], ct["LT_w2c"][:, bass.ts(j, 32)],
                        h1s_tiles[GS * g + j][:],
                        start=True, stop=True, skip_group_check=True,
                        tile_position=(0, 32 * j),
                    )
                og = og_pool.tile([H, BT], F32, name="og")
                nc.vector.tensor_tensor(og[:], wp, t2g[:], mybir.AluOpType.mult)
                if not zero_bias:
                    og2 = og_pool.tile([H, BT], F32, name="og2")
                    nc.scalar.activation(og2[:], og[:], AF.Identity, bias=ct["B2r"][:])
                    og = og2
                for s in range(4):
                    qs[s].dma_start(
                        outT[:, bass.ts(GS * g + s, BT)], og[bass.ts(s, 32), :]
                    )

    if split_waits:
        _split_multi_waits(nc)
    return nc


def _host_params(G, W_se1, b_se1, W_se2, b_se2, W1, b1, W2, b2, nt):
    import ml_dtypes
    f = np.float32
    bf = ml_dtypes.bfloat16
    ngrp = nt // GS
    G = np.asarray(G, f)
    Gflat = np.transpose(G, (0, 2, 1)).reshape(NG * D, D)  # [(g,i), j] = G[g,j,i]
    W1G = Gflat @ np.asarray(W1, f).T                      # [(g,i), m]
    I32 = np.eye(D, dtype=f)

    onsq = np.zeros((H, 4, 32), f)
    for r in range(4):
        onsq[:, r, r] = 1.0
    E4 = np.zeros((H, ngrp, H), f)
    for g in range(ngrp):
        for r in range(GS):
            E4[32 * r + g, g, 32 * r:32 * (r + 1)] = 0.5

    p = {
        "LT_h4": np.tile(np.asarray(W_se1, f).T, (4, 1)),
        "LT_At": np.repeat(np.asarray(W_se2, f).T[:, 0:4], 32, axis=1),
        "LT_Ab": np.repeat(np.asarray(W_se2, f).T[:, 4:8], 32, axis=1),
        "Bse1": np.asarray(b_se1, f).reshape(H, 1),
        "Bse2t": np.repeat(np.asarray(b_se2, f)[0:4], 32).reshape(H, 1),
        "Bse2b": np.repeat(np.asarray(b_se2, f)[4:8], 32).reshape(H, 1),
        "LT_z4": np.tile(I32, (4, 4)),
        "LT_w1z4": np.tile(np.asarray(W1, f).T, (4, 1)),
        "B1": np.asarray(b1, f).reshape(H, 1),
        "LT_t1": np.ascontiguousarray(W1G[:H]),
        "LT_b1": np.ascontiguousarray(W1G[H:]),
        "onsq4": onsq.reshape(H, H),
        "E4": E4.reshape(H, ngrp * H),
        "LT_w2c": np.tile(np.asarray(W2, f).T, (1, 4)),
        "B2r": np.tile(np.asarray(b2, f), 4).reshape(H, 1),
    }
    for k, tname, bname in ((4, "LT_t4", "LT_b4"), (3, "LT_t3", "LT_b3"), (2, "LT_t2", "LT_b2")):
        scaled = np.tile(Gflat * f(1.0 / k), (1, 4))
        p[tname] = np.ascontiguousarray(scaled[:H])
        p[bname] = np.ascontiguousarray(scaled[H:])
    for name, (shape, dt) in _PARAM_SHAPES.items():
        assert list(p[name].shape) == shape, (name, p[name].shape, shape)
        if dt == BF16:
            p[name] = p[name].astype(bf)
        else:
            p[name] = np.ascontiguousarray(p[name], f)
    return p


def _run(z, G, W_se1, b_se1, W_se2, b_se2, W1, b1, W2, b2, trace=False, **trace_kw):
    import ml_dtypes
    z = np.asarray(z, np.float32)
    nt = BC // BT
    # b_se1/b1 go through ACT bias slots either way; only b_se2/b2 change
    # the instruction count.
    zero_bias = (float(np.abs(np.asarray(b_se2)).max()) == 0.0
                 and float(np.abs(np.asarray(b2)).max()) == 0.0)
    params = _host_params(G, W_se1, b_se1, W_se2, b_se2, W1, b1, W2, b2, nt)

    zT = np.ascontiguousarray(
        z.reshape(NCORES, BC, D).transpose(0, 2, 1)
    ).astype(ml_dtypes.bfloat16)

    nc = _build_program(BC, zero_bias)
    in_maps = [{"zT": zT[c], **params} for c in range(NCORES)]
    res = run_bass_kernel_spmd(nc, in_maps, list(range(NCORES)), trace=trace, **trace_kw)

    outT = np.stack([res.results[c]["outT"] for c in range(NCORES)])
    out = outT.transpose(0, 2, 1).reshape(B, D)
    return np.ascontiguousarray(out.astype(np.float32)), res


def kernel(z, G, W_se1, b_se1, W_se2, b_se2, W1, b1, W2, b2):
    out, _ = _run(z, G, W_se1, b_se1, W_se2, b_se2, W1, b1, W2, b2, trace=False)
    return out


if __name__ == "__main__":
    rng = np.random.default_rng(0)
    inputs = {
        "z": rng.standard_normal((B, D), dtype=np.float32),
        "G": (rng.standard_normal((NG, D, D)) * 0.1).astype(np.float32),
        "W_se1": (rng.standard_normal((H, D)) / np.sqrt(D)).astype(np.float32),
        "b_se1": np.zeros(H, np.float32),
        "W_se2": (rng.standard_normal((NG, H)) / np.sqrt(H)).astype(np.float32),
        "b_se2": np.zeros(NG, np.float32),
        "W1": (rng.standard_normal((H, D)) * 0.01).astype(np.float32),
        "b1": np.zeros(H, np.float32),
        "W2": (rng.standard_normal((D, H)) * 0.01).astype(np.float32),
        "b2": np.zeros(D, np.float32),
    }
    out = kernel(**inputs)
    print("kernel output", out.shape, out.dtype, float(np.abs(out).max()))
